# revision 8
# baseline (speedup 1.0000x reference)
"""Trainium2 Bass kernel for nn_Class1ProcessingModel (ragged peptide model).

Self-contained: takes FULL inputs (as produced by setup_inputs), shards the
batch over 8 NeuronCores, runs a Bass/Tile kernel via run_bass_kernel_spmd,
and gathers the full [B] output.

Structure (see kernel_baseline.py.bak for the bf16 ancestor):
  - conv as fp8 DoubleRow matmuls: the 189-row windows fit 2x128 k-tiles,
    one DoubleRow pass per position at 0.5 cyc/row.  Window alignment via
    two SBUF copies of the features (chunked at flat row 0 and 64);
    sub-chunk offsets handled with zero-padded weight rows.  A second pass
    with the e5m2 weight residual cancels the weight-quantization error
    (~1.15e-2 final rel err vs 1.7e-2 plain; gate is 2e-2) and hides under
    the drain engines.
  - PSUM->SBUF relu drains are the real bottleneck (~1 col/cycle on each of
    ACT and DVE; GPSIMD has no PSUM port), so they are explicitly
    load-balanced between ACT and DVE.
  - all ragged masks are precomputed on the HOST from peptide_length and
    DMA'd as bf16 tables -- no on-device mask/iota/prep work at all.
  - per-row scalars land batch-on-partitions in PSUM via data-stationary
    tiny matmuls; st layout puts the final 6-vector at cols 0..5 directly.
  - the finale of tile t is emitted during tile t+1 (software pipelining)
    so the drain engines never sit behind it; x tiles are double-buffered
    with explicit cross-iteration prefetch.
"""

import os
import numpy as np
import ml_dtypes

B, L, CIN = 32768, 35, 21
N_FLANK, C_FLANK, PEP_MAX = 10, 10, 15
F, K, H = 128, 9, 64
N_CORES = 8
BSH = B // N_CORES          # rows per core
NB = 512                    # batch-tile (matmul moving free dim)
NT = BSH // NB              # batch tiles per core
NG = NB // 128              # 128-row groups per batch tile
NGT = BSH // 128            # groups per core (= NT*NG)
FLAT = L * CIN              # 735
NCH = 6                     # feature chunks of 128
FPAD = NCH * 128            # 768, padded flat features
BB = 1024.0                 # masked-max offset, folded out via b6
SC = 56                     # st cols per group: 6 t6 | 30 z | 20 qc

# conv drain order: branch positions first so branch1 can start early
ORDER = list(range(10, 25)) + list(range(0, 10)) + list(range(25, 35))
COL = {l: i for i, l in enumerate(ORDER)}   # cr column-block for position l

_CACHE = {}


def _conv_plan():
    """Per output position: (copy, c) with copy 0 chunked at flat row 0 and
    copy 1 at flat row 64.  The 189-row window [lo, hi) sits in chunks
    (c, c+1) of the chosen copy, starting at in-chunk offset <= 67."""
    plan = []
    for l in range(L):
        s = CIN * (l - K // 2)
        lo, hi = max(0, s), min(FLAT, s + K * CIN)
        if lo % 128 + (hi - lo) <= 256:
            copy, c = 0, lo // 128
        else:
            copy, c = 1, (lo - 64) // 128
        off = (lo - 64 * copy) % 128
        assert off + (hi - lo) <= 256 and c + 1 < NCH, (l, off, lo, hi)
        plan.append((copy, c, s, lo, hi))
    return plan


def _build_program(repeat=1):
    import contextlib
    import concourse.bass as bass
    import concourse.mybir as mybir
    import concourse.tile as tile

    dt = mybir.dt
    AF = mybir.ActivationFunctionType
    OP = mybir.AluOpType
    DR = mybir.MatmulPerfMode.DoubleRow
    plan = _conv_plan()

    nc = bass.Bass()
    xa_d = nc.declare_dram_parameter("xa", [NT * 128, NCH * NB], dt.float8e4, isOutput=False)
    xb_d = nc.declare_dram_parameter("xb", [NT * 128, NCH * NB], dt.float8e4, isOutput=False)
    wh_d = nc.declare_dram_parameter("wh", [128, L * 256], dt.float8e4, isOutput=False)
    wl_d = nc.declare_dram_parameter("wl", [128, L * 256], dt.float8e5, isOutput=False)
    w1_d = nc.declare_dram_parameter("w1", [128, 128], dt.bfloat16, isOutput=False)
    w2_d = nc.declare_dram_parameter("w2", [128, 2], dt.bfloat16, isOutput=False)
    vcat_d = nc.declare_dram_parameter("vcat", [128, 2], dt.bfloat16, isOutput=False)
    convb_d = nc.declare_dram_parameter("convb", [128, 1], dt.float32, isOutput=False)
    b1_d = nc.declare_dram_parameter("b1", [128, 1], dt.float32, isOutput=False)
    mn_d = nc.declare_dram_parameter("mn", [128, NGT * 15], dt.bfloat16, isOutput=False)
    mc_d = nc.declare_dram_parameter("mc", [128, NGT * 15], dt.bfloat16, isOutput=False)
    oh_d = nc.declare_dram_parameter("oh", [128, NGT * 15], dt.bfloat16, isOutput=False)
    mw_d = nc.declare_dram_parameter("mw", [128, NGT * 20], dt.bfloat16, isOutput=False)
    b6_d = nc.declare_dram_parameter("b6", [128, NG * 6], dt.float32, isOutput=False)
    w6_d = nc.declare_dram_parameter("w6", [128, NG * 6], dt.float32, isOutput=False)
    out_d = nc.declare_dram_parameter("out", [128, NGT], dt.float32, isOutput=True)
    out_bias = float(_CACHE["out_b"])

    # running drain-engine load balance (est. ns)
    eng_load = {"act": 0.0, "dve": 0.0}

    with tile.TileContext(nc) as tc:
        with (
            tc.tile_pool(name="persist", bufs=1) as pp,
            tc.tile_pool(name="cr", bufs=2) as crp,
            tc.tile_pool(name="hid", bufs=2) as hidp,
            tc.tile_pool(name="wk", bufs=2) as wkp,
            tc.tile_pool(name="cvps", bufs=3, space="PSUM") as cvps,
            tc.tile_pool(name="stps", bufs=2, space="PSUM") as stps,
        ):
            # ---- persistent tiles ----
            xt = [(pp.tile([128, NCH * NB], dt.float8e4, tag=f"xa{i}", name=f"xa{i}"),
                   pp.tile([128, NCH * NB], dt.float8e4, tag=f"xb{i}", name=f"xb{i}"))
                  for i in range(2)]
            wh = pp.tile([128, L * 256], dt.float8e4, tag="wh")
            wl = pp.tile([128, L * 256], dt.float8e5, tag="wl")
            w1 = pp.tile([128, 128], dt.bfloat16, tag="w1")
            w2 = pp.tile([128, 2], dt.bfloat16, tag="w2")
            vcat = pp.tile([128, 2], dt.bfloat16, tag="vcat")
            convb = pp.tile([128, 1], dt.float32, tag="convb")
            b1 = pp.tile([128, 1], dt.float32, tag="b1")
            mn = pp.tile([128, NGT * 15], dt.bfloat16, tag="mn")
            mc = pp.tile([128, NGT * 15], dt.bfloat16, tag="mc")
            oh = pp.tile([128, NGT * 15], dt.bfloat16, tag="oh")
            mw = pp.tile([128, NGT * 20], dt.bfloat16, tag="mw")
            b6 = pp.tile([128, NG * 6], dt.float32, tag="b6")
            w6 = pp.tile([128, NG * 6], dt.float32, tag="w6")
            osb = pp.tile([128, NGT], dt.float32, tag="osb")

            # x tiles for tiles 0/1 first so compute starts ASAP, then the
            # conv consts, then everything the finale needs.
            def load_x(t):
                xa, xb = xt[t % 2]
                nc.sync.dma_start(xa[:], xa_d[(t % NT) * 128:(t % NT) * 128 + 128, :])
                nc.sync.dma_start(xb[:], xb_d[(t % NT) * 128:(t % NT) * 128 + 128, :])

            load_x(0)
            load_x(1)
            for tdst, tsrc in [(wh, wh_d), (wl, wl_d), (convb, convb_d),
                               (w1, w1_d), (b1, b1_d), (w2, w2_d),
                               (vcat, vcat_d), (mn, mn_d), (mc, mc_d),
                               (oh, oh_d), (mw, mw_d), (b6, b6_d), (w6, w6_d)]:
                nc.sync.dma_start(tdst[:], tsrc[:])

            whv = wh[:].rearrange("p (l i f) -> p l i f", i=2, f=128)
            wlv = wl[:].rearrange("p (l i f) -> p l i f", i=2, f=128)
            b6v = b6[:].rearrange("p (g c) -> p g c", c=6)
            mnv = mn[:].rearrange("p (g c) -> p g c", c=15)
            mcv = mc[:].rearrange("p (g c) -> p g c", c=15)
            ohv = oh[:].rearrange("p (g c) -> p g c", c=15)
            mwv = mw[:].rearrange("p (g c) -> p g c", c=20)

            def drain(dst, src, bias, width):
                """PSUM->SBUF relu+bias on the lighter of ACT/DVE."""
                cact = (width + 222) * 0.833
                cdve = (width + 120) * 1.042
                if eng_load["act"] + cact <= eng_load["dve"] + cdve:
                    eng_load["act"] += cact
                    nc.scalar.activation(dst, src, AF.Relu, bias=bias)
                else:
                    eng_load["dve"] += cdve
                    nc.vector.tensor_scalar(dst, src, bias, 0.0, OP.add, OP.max)

            def finale(t, st):
                """Emit the ragged tail for tile t (st is its PSUM tile)."""
                stv = st[:].rearrange("p (g c) -> p g c", c=SC)
                zn = stv[:, :, 6:36:2]
                zc = stv[:, :, 7:37:2]
                qc = stv[:, :, 36:56]
                g0 = slice(t * NG, (t + 1) * NG)

                def w3(tag, w=15):
                    return wkp.tile([128, NG * w], dt.float32, tag=tag, name=tag)[:].rearrange(
                        "p (g c) -> p g c", c=w)

                # masked max via (z + BB)*mask, BB folded out through b6
                tz = w3("tz")
                nc.vector.scalar_tensor_tensor(tz, zn, BB, mnv[:, g0], OP.add, OP.mult)
                nc.vector.reduce_max(stv[:, :, 1:2], tz, axis=mybir.AxisListType.X)
                tz2 = w3("tz2")
                nc.vector.scalar_tensor_tensor(tz2, zc, BB, mcv[:, g0], OP.add, OP.mult)
                nc.vector.reduce_max(stv[:, :, 4:5], tz2, axis=mybir.AxisListType.X)
                sel = w3("sel")
                nc.vector.tensor_tensor(sel, zc, ohv[:, g0], OP.mult)
                nc.vector.reduce_sum(stv[:, :, 3:4], sel, axis=mybir.AxisListType.X)
                qt = w3("qt", 20)
                nc.vector.tensor_tensor(qt, qc, mwv[:, g0], OP.mult)
                nc.vector.reduce_sum(stv[:, :, 5:6], qt, axis=mybir.AxisListType.X)

                # biases, tanh, output weights, sigmoid
                t6 = wkp.tile([128, NG * 6], dt.float32, tag="t6")
                nc.vector.tensor_tensor(t6[:].rearrange("p (g c) -> p g c", c=6),
                                        stv[:, :, 0:6], b6v, OP.add)
                t6b = wkp.tile([128, NG * 6], dt.float32, tag="t6b")
                nc.scalar.activation(t6b[:, :], t6[:, :], AF.Tanh)
                nc.gpsimd.tensor_tensor(t6b[:, :], t6b[:, :], w6[:, :], OP.mult)
                s1 = wkp.tile([128, NG], dt.float32, tag="s1")
                nc.vector.reduce_sum(s1[:, :], t6b[:].rearrange("p (g c) -> p g c", c=6),
                                     axis=mybir.AxisListType.X)
                nc.scalar.activation(osb[:, t * NG:(t + 1) * NG], s1[:, :],
                                     AF.Sigmoid, bias=out_bias)

            rep_ctx = tc.For_i(0, repeat, 1) if repeat > 1 else contextlib.nullcontext()
            with rep_ctx:
              prev = None
              for t in range(NT):
                xa, xb = xt[t % 2]
                xav = xa[:].rearrange("p (c j) -> p c j", c=NCH)
                xbv = xb[:].rearrange("p (c j) -> p c j", c=NCH)

                # ---- conv: one hi+lo DoubleRow pair per output position;
                # 2 positions per psum tile ----
                # account the fixed finale work in the drain balance
                eng_load["dve"] += 1900.0
                eng_load["act"] += 500.0
                cr = crp.tile([128, L * NB], dt.bfloat16, tag="cr")
                cv = None
                for oi, l in enumerate(ORDER):
                    half = oi % 2
                    if half == 0:
                        cv = cvps.tile([128, 2 * NB], dt.float32, tag="cv")
                    copy, c, s, lo, hi = plan[l]
                    rhs = (xbv if copy else xav)[:, c:c + 2, :]
                    dst = cv[:, half * NB:(half + 1) * NB]
                    nc.tensor.matmul(dst, lhsT=whv[:, l], rhs=rhs,
                                     start=True, stop=False, perf_mode=DR)
                    nc.tensor.matmul(dst, lhsT=wlv[:, l], rhs=rhs,
                                     start=False, stop=True, perf_mode=DR)
                    if half == 1 or oi == L - 1:
                        w = (half + 1) * NB
                        drain(cr[:, (oi - half) * NB:(oi - half) * NB + w],
                              cv[:, 0:w], convb[:, 0:1], w)

                # ---- branch layer 1 (both branches fused), bf16 ----
                hid = hidp.tile([128, 15 * NB], dt.bfloat16, tag="hid")
                for i, l in enumerate(range(10, 25)):
                    hp = cvps.tile([128, 2 * NB], dt.float32, tag="cv", name="hp")[:, 0:NB]
                    nc.tensor.matmul(hp[:, :], lhsT=w1[:, :],
                                     rhs=cr[:, COL[l] * NB:(COL[l] + 1) * NB],
                                     start=True, stop=True)
                    drain(hid[:, i * NB:(i + 1) * NB], hp[:, :], b1[:, 0:1], NB)

                # ---- per-row scalars (batch on partitions) ----
                # st cols per group: 0..5 = t6 [zn0, maxn, navg, selc, maxc,
                # qcsum], 6..35 = z_n/z_c pairs, 36..55 = qc projections.
                st = stps.tile([128, NG * SC], dt.float32, tag="st")
                for g in range(NG):
                    sb = g * SC
                    nc.tensor.matmul(st[:, sb:sb + 1],
                                     lhsT=hid[:, g * 128:(g + 1) * 128],
                                     rhs=w2[:, 0:1], start=True, stop=True)
                    for i in range(15):
                        nc.tensor.matmul(
                            st[:, sb + 6 + 2 * i:sb + 8 + 2 * i],
                            lhsT=hid[:, i * NB + g * 128:i * NB + (g + 1) * 128],
                            rhs=w2[:, 0:2], start=True, stop=True)
                    for j, l in enumerate(range(15, 35)):
                        nc.tensor.matmul(
                            st[:, sb + 36 + j:sb + 37 + j],
                            lhsT=cr[:, COL[l] * NB + g * 128:COL[l] * NB + (g + 1) * 128],
                            rhs=vcat[:, 1:2], start=True, stop=True)
                    for j, l in enumerate(range(0, 10)):
                        nc.tensor.matmul(
                            st[:, sb + 2:sb + 3],
                            lhsT=cr[:, COL[l] * NB + g * 128:COL[l] * NB + (g + 1) * 128],
                            rhs=vcat[:, 0:1], start=(j == 0), stop=(j == 9))

                # prefetch x for tile t+2 (wraps across repeat iterations)
                load_x(t + 2)

                # ---- previous tile's ragged finale (pipelined) ----
                if prev is not None:
                    finale(*prev)
                prev = (t, st)

              finale(*prev)

            nc.sync.dma_start(out_d[:], osb[:])

    _split_excess_waits(nc)
    return nc


def _split_excess_waits(nc, max_waits=1):
    """This walrus build rejects instructions carrying multiple sync waits
    (the TileContext tail drain gets the whole global clock attached).
    Move excess waits onto injected same-engine NoOps just before."""
    import concourse.mybir as mybir
    for f in nc.m.functions:
        for bb in f.blocks:
            out, changed = [], False
            for inst in bb.instructions:
                si = inst.sync_info
                waits = list(si.on_wait) if si and si.on_wait else []
                if len(waits) > max_waits:
                    extra, keep = waits[:-max_waits], waits[-max_waits:]
                    for i in range(0, len(extra), max_waits):
                        nop = mybir.InstNoOp(name=f"{inst.name}-wsplit-{i}",
                                             ins=[], outs=[])
                        nop.engine = inst.engine
                        nop.sync_info = mybir.SyncInfo(
                            on_wait=extra[i:i + max_waits], on_update=[])
                        out.append(nop)
                    inst.sync_info = mybir.SyncInfo(
                        on_wait=keep,
                        on_update=list(si.on_update) if si.on_update else [])
                    changed = True
                out.append(inst)
            if changed:
                bb.instructions = out


def _host_consts(conv_w, conv_b, n_w1, n_b1, n_w2, n_b2, c_w1, c_b1, c_w2,
                 c_b2, navg_w, navg_b, cavg_w, cavg_b, out_w, out_b):
    bf16 = ml_dtypes.bfloat16
    e4 = ml_dtypes.float8_e4m3
    e5 = ml_dtypes.float8_e5m2
    plan = _conv_plan()
    wflat = np.asarray(conv_w, np.float32).reshape(K * CIN, F)
    # DoubleRow conv weights: [128 part, l, ktile i, f] with zero padding
    # outside the window; hi = e4m3 quant, lo = e5m2 residual.
    wfull = np.zeros((128, L, 2, F), np.float32)
    for l in range(L):
        copy, c, s, lo, hi = plan[l]
        base = 64 * copy
        for i in range(2):
            r0 = base + 128 * (c + i)            # flat row of partition 0
            a = max(lo, r0)
            bz = min(hi, r0 + 128)
            if bz > a:
                wfull[a - r0:bz - r0, l, i, :] = wflat[a - s:bz - s, :]
    whq = wfull.astype(e4)
    wlq = (wfull - whq.astype(np.float32)).astype(e5)

    w1 = np.concatenate([np.asarray(n_w1, np.float32),
                         np.asarray(c_w1, np.float32)], axis=1)      # [128,128]
    w2 = np.zeros((128, 2), np.float32)
    w2[0:H, 0] = np.asarray(n_w2, np.float32)[:, 0]
    w2[H:128, 1] = np.asarray(c_w2, np.float32)[:, 0]
    vcat = np.stack([np.asarray(navg_w, np.float32)[:, 0] / N_FLANK,
                     np.asarray(cavg_w, np.float32)[:, 0] / C_FLANK], axis=1)
    b1cat = np.concatenate([np.asarray(n_b1, np.float32),
                            np.asarray(c_b1, np.float32)])[:, None]
    rep = lambda row: np.tile(np.asarray(row, np.float32)[None, :], (128, NG)).copy()
    ow = np.asarray(out_w, np.float32)[:, 0] * np.array([1, -1, 1, 1, -1, 1], np.float32)
    sc = lambda x: float(np.asarray(x).reshape(-1)[0])
    b6 = np.array([sc(n_b2), sc(n_b2) - BB, sc(navg_b),
                   sc(c_b2), sc(c_b2) - BB, sc(cavg_b)], np.float32)
    return {
        "wh": np.ascontiguousarray(whq.reshape(128, L * 2 * F)),
        "wl": np.ascontiguousarray(wlq.reshape(128, L * 2 * F)),
        "w1": w1.astype(bf16),
        "w2": w2.astype(bf16),
        "vcat": vcat.astype(bf16),
        "convb": np.asarray(conv_b, np.float32)[:, None].copy(),
        "b1": b1cat.copy(),
        "b6": rep(b6),
        "w6": rep(ow),
    }, sc(out_b)


def _pack_x(seq_core):
    """[BSH, FPAD] fp32 -> (xa, xb) [NT*128, NCH*NB] e4m3, chunk-major per
    batch tile: row t*128+p, col c*NB+j  =  x[t*NB+j, 128c+p] (copy A) or
    x[t*NB+j, 64+128c+p] (copy B, zero-padded past FPAD)."""
    e4 = ml_dtypes.float8_e4m3
    xq = seq_core.astype(e4)
    xbsrc = np.zeros_like(xq)
    xbsrc[:, :FPAD - 64] = xq[:, 64:]
    packs = []
    for src in (xq, xbsrc):
        p = src.reshape(NT, NB, NCH, 128).transpose(0, 3, 2, 1)
        packs.append(np.ascontiguousarray(p.reshape(NT * 128, NCH * NB)))
    return packs


def _masks(plen_core):
    """Host-precomputed ragged masks, [128, NGT*w] bf16 group-major."""
    bf16 = ml_dtypes.bfloat16
    p = plen_core.reshape(NGT, 128, 1).astype(np.float32)
    i15 = np.arange(15, dtype=np.float32)[None, None, :]
    ion = np.broadcast_to(i15, (1, 1, 15)).copy(); ion[0, 0, 0] = 1e9
    mn = (ion < p)                        # max_n slots: 1 <= i < plen
    mc = (i15 + 1.0 < p)                  # max_c slots: i < plen-1
    ohm = (i15 + 1.0 == p)                # cleaved_c one-hot: i == plen-1
    j20 = np.arange(20, dtype=np.float32)[None, None, :]
    mwin = (j20 >= p - 5.0) & (j20 < p + 5.0)   # c-flank window slots
    pack = lambda m: np.ascontiguousarray(
        m.astype(bf16).transpose(1, 0, 2).reshape(128, -1))
    return pack(mn), pack(mc), pack(ohm), pack(mwin)


def _make_in_maps(inputs):
    seq = np.asarray(inputs["sequence"], np.float32)
    plen = np.asarray(inputs["peptide_length"], np.int32)
    consts, out_bias = _host_consts(
        inputs["conv_w"], inputs["conv_b"], inputs["n_w1"], inputs["n_b1"],
        inputs["n_w2"], inputs["n_b2"], inputs["c_w1"], inputs["c_b1"],
        inputs["c_w2"], inputs["c_b2"], inputs["navg_w"], inputs["navg_b"],
        inputs["cavg_w"], inputs["cavg_b"], inputs["out_w"], inputs["out_b"])
    _CACHE["out_b"] = out_bias

    seq_flat = np.zeros((B, FPAD), np.float32)
    seq_flat[:, :FLAT] = seq.reshape(B, FLAT)

    in_maps = []
    for i in range(N_CORES):
        sh = slice(i * BSH, (i + 1) * BSH)
        m = dict(consts)
        m["xa"], m["xb"] = _pack_x(seq_flat[sh])
        m["mn"], m["mc"], m["oh"], m["mw"] = _masks(plen[sh])
        in_maps.append(m)
    return in_maps


def kernel(**inputs):
    from concourse.bass_utils import run_bass_kernel_spmd

    in_maps = _make_in_maps(inputs)
    if "nc" not in _CACHE:
        _CACHE["nc"] = _build_program()
    nc = _CACHE["nc"]

    trace = bool(int(os.environ.get("TRN_KERNEL_TRACE", "0")))
    res = run_bass_kernel_spmd(nc, in_maps, list(range(N_CORES)), trace=trace)
    if trace and res.exec_time_ns is not None:
        print(f"HW exec time: {res.exec_time_ns} ns")
        _CACHE["exec_time_ns"] = res.exec_time_ns
        _CACHE["profile"] = res

    out = np.empty((B,), np.float32)
    for i in range(N_CORES):
        arr = np.asarray(res.results[i]["out"], np.float32)   # [128, NGT]
        out[i * BSH:(i + 1) * BSH] = arr.T.reshape(-1)
    return out
